# revision 5
# baseline (speedup 1.0000x reference)
"""Trainium2 Bass kernel for nn_Blur: 4x4 FIR depthwise blur with pad (2,1).

out[n,c,i,j] = sum_{a,b} K[a,b] * x[n,c, i+1-a, j+1-b]   (zero-padded)

Strategy (8 NeuronCores, pure data parallelism over the 8192 (n,c) slices):
  - fp16 end-to-end on device (host converts): halves HBM traffic vs fp32.
    Quantization error ~5e-4 relative, far under the 2e-2 gate.
  - Main path (13 of 16 tiles): w-parity interleaved layout, partition
    p = 64*(w%2) + h; free dim packs each slice as [zero-block][32 data
    w-blocks] (33 cols), zero blocks shared between neighbours. The 16-tap
    conv is THREE PSUM-accumulated matmuls (free-dim block shifts
    d in {-1,0,+1}): lhsT_d[(jp_in,u),(jp_out,i)] = K[i-u+1, jp_out-jp_in
    +1-2d]. 12 x N=512 columns per 64-slice tile.
  - Offload path (3 tiles): separable FIR. DVE+GPSIMD do the W-conv
    (t1 = x<<0 + x<<3; t2 = x<<1 + x<<2; y2 = 3*t2 + t1) on a (member,h)
    partition layout, then ONE matmul pass applies the h-band k1/16.
    This pulls the PE (41.5us) under the fp16 DMA span (~41us) so the
    stream is DMA-bound, not PE-bound.
  - Startup: the HAM clock gate needs ~4096 cycles (3.4us) of observed PE
    activity to open (1.2 -> 2.4 GHz), and ANY idle gap resets it. Eight
    junk matmuls on a memset tile burn a full window unconditionally, so
    real matmuls start warm and never re-throttle. Tile 0 is split in
    halves so its data beats the PE to the handoff.
  - Drain: last tile stores per-PSUM-group on the ACT ring; the final
    group's copy runs on DVE alone and its store goes down the by-then-idle
    SP ring immediately.
"""

import sys
import types

import numpy as np

import concourse.bacc as bacc
import concourse.mybir as mybir
from concourse.alu_op_type import AluOpType
from concourse.tile import TileContext
from concourse.bass_utils import run_bass_kernel_spmd


def _install_ntff_hook():
    """Best-effort shim: this image's antenv lacks axon_hooks, which the
    trace=True path of run_bass_kernel_spmd imports. Harmless if unused."""
    if "antenv.axon_hooks" in sys.modules:
        return
    try:
        sys.path.insert(0, "/root/.axon_site")
        from trn_agent_boot.trn_boot import _ntff_profile_via_ctypes

        hook = _ntff_profile_via_ctypes("/opt/axon/libaxon_pjrt.so")
        mod = types.ModuleType("antenv.axon_hooks")
        mod.get_axon_ntff_profile_hook = lambda: hook
        mod.set_axon_ntff_profile_hook = lambda h: None
        sys.modules["antenv.axon_hooks"] = mod
    except Exception:
        pass


_install_ntff_hook()

N_CORES = 8
B, C, H, W = 32, 256, 64, 64
NSLICES = B * C                      # 8192
SLICES_PER_CORE = NSLICES // N_CORES  # 1024
TILE_SLICES = 64                     # slices per full SBUF tile
JB = W // 2                          # 32 w-blocks of 2 per slice
FREE = TILE_SLICES * JB              # 2048: NO padding (edge-skip matmuls)
GQ = 16                              # slices per PSUM group (N = 16*32 = 512)
GF = GQ * JB                         # free columns per group = 512
WP = W + 3                           # offload path: 2 left + 1 right zero
SG = TILE_SLICES // 2                # offload path: s-groups per member
F16 = mybir.dt.float16
F32 = mybir.dt.float32

# Separable DVE/GPS offload measured SLOWER in aggregate (engines run at
# ~75-115 G elem/s, half the spec sheet): the W-conv chain can't stay ahead
# of the PE and every slip resets the HAM clock gate. Keep it off.
OFFLOAD = ()
WARMUP_MMS = 4                       # junk matmuls until first data lands

_NC_CACHE = {}


def _build_wmat(K: np.ndarray) -> np.ndarray:
    """[128, 4*128] fp16: lhsT stack [d=0, d=-1, d=+1, h-band k1/16]."""
    K = np.asarray(K, np.float32)
    wmat = np.zeros((4, 128, 128), np.float32)
    for di, d in enumerate((0, -1, 1)):
        L = wmat[di]
        for jpi in range(2):
            for jpo in range(2):
                b = jpo - jpi + 1 - 2 * d
                if not (0 <= b < 4):
                    continue
                for i in range(H):
                    for a in range(4):
                        u = i + 1 - a
                        if 0 <= u < H:
                            L[64 * jpi + u, 64 * jpo + i] += K[a, b]
    # h-band for the separable path: lhsT[u+64m, i+64m] = k1[i-u+1]/16
    k1 = np.array([1.0, 3.0, 3.0, 1.0], np.float32) / 16.0
    T = np.zeros((H, H), np.float32)
    for i in range(H):
        for a in range(4):
            u = i + 1 - a
            if 0 <= u < H:
                T[u, i] += k1[a]
    wmat[3, :H, :H] = T
    wmat[3, H:, H:] = T
    # [d, k, m] -> [k, (d m)] so the DMA is one contiguous run per partition
    return np.ascontiguousarray(
        wmat.transpose(1, 0, 2).reshape(128, 4 * 128)
    ).astype(np.float16)


def _build_nc(slices_per_core: int = SLICES_PER_CORE):
    ntiles = slices_per_core // TILE_SLICES
    nc = bacc.Bacc("TRN2", target_bir_lowering=False, debug=False)
    x = nc.dram_tensor(
        "x", [ntiles, 128, FREE], F16, kind="ExternalInput"
    ).ap()
    xo = (
        nc.dram_tensor(
            "xo", [len(OFFLOAD), 128, SG * WP], F16, kind="ExternalInput"
        ).ap()
        if OFFLOAD
        else None
    )
    wm = nc.dram_tensor("w", [128, 4 * 128], F16, kind="ExternalInput").ap()
    y = nc.dram_tensor(
        "y", [ntiles, 128, TILE_SLICES * JB], F16, kind="ExternalOutput"
    ).ap()
    # sink for the PE warm-up matmuls (kept alive so DCE can't drop them)
    warm_out = nc.dram_tensor("warm", [128, 4], F32, kind="ExternalOutput").ap()

    # main-path chunk = (dram tile, first group, n groups); tile 0 is
    # split 1+1+2 so the first 131KB lands ~8.6us (right as warmup ends)
    # and each later piece's receipt hides under earlier matmuls.
    chunks = [(0, 0, 1), (0, 1, 1), (0, 2, 2)]
    chunks += [(t, 0, 4) for t in range(1, ntiles) if t not in OFFLOAD]
    last = len(chunks) - 1

    with TileContext(nc) as tc:
        with (
            tc.tile_pool(name="wpool", bufs=1) as wpool,
            tc.tile_pool(name="xpool", bufs=8) as xpool,
            tc.tile_pool(name="vpool", bufs=4) as vpool,
            tc.tile_pool(name="opool", bufs=6) as opool,
            tc.tile_pool(name="pspool", bufs=8, space="PSUM") as pspool,
        ):
            # weight tile: its DMA is issued FIRST on the SP ring -- the
            # first real matmul needs weights AND data, and weights (131KB)
            # cost tile 0's first group only ~0.4us of extra ring time,
            # while riding after it cost ~2us (landed 10.7us, not 8.3us).
            wsb = wpool.tile([128, 4, 128], F16, name="wsb")

            # HAM warm-up: junk matmuls (no DMA dependency) open the clock
            # gate (~4.5us of observed PE activity) before real work. The
            # memset runs on DVE, which clears its preamble earliest.
            wjunk = wpool.tile([128, 512], F16, name="wjunk")
            nc.vector.memset(wjunk[:], 0.0)
            wscratch = wpool.tile([128, 4], F32, name="wscratch")
            wps = pspool.tile([128, 512], F32, name="wps", tag="ps")
            for r in range(WARMUP_MMS):
                nc.tensor.matmul(
                    wps[:],
                    wjunk[:, 0:128],
                    wjunk[:],
                    start=(r == 0),
                    stop=(r == WARMUP_MMS - 1),
                )
            nc.vector.tensor_copy(wscratch[:], wps[:, 0:4])
            nc.scalar.dma_start(warm_out, wscratch[:])

            oi = {t: i for i, t in enumerate(OFFLOAD)}
            ncopy = 0

            def offload_tile(t):
                """Separable path: W-conv on DVE/GPS, one h-band PE pass."""
                xt = xpool.tile([128, SG, WP], F16, name="xof")
                nc.sync.dma_start(xt[:], xo[oi[t]])
                t1 = vpool.tile([128, SG, W], F16, name="t1")
                t2 = vpool.tile([128, SG, W], F16, name="t2")
                y2 = vpool.tile([128, SG, W], F16, name="y2")
                # both adds on GPSIMD (SBUF-only ops are Pool-legal); the
                # fused 3*t2+t1 is DVE-only (TensorScalarPtr not on Pool)
                nc.gpsimd.tensor_tensor(
                    t1[:], xt[:, :, 0:W], xt[:, :, 3 : 3 + W], AluOpType.add
                )
                nc.gpsimd.tensor_tensor(
                    t2[:], xt[:, :, 1 : 1 + W], xt[:, :, 2 : 2 + W],
                    AluOpType.add,
                )
                nc.vector.scalar_tensor_tensor(
                    y2[:], t2[:], 3.0, t1[:],
                    op0=AluOpType.mult, op1=AluOpType.add,
                )
                ot = opool.tile([128, SG, W], F16, name="ot")
                for q in range(4):
                    ps = pspool.tile([128, GQ * JB], F32, name="ps")
                    nc.tensor.matmul(
                        ps[:], wsb[:, 3, :], y2[:, 8 * q : 8 * (q + 1), :],
                        start=True, stop=True,
                    )
                    dst = ot[:, 8 * q : 8 * (q + 1), :]
                    if q % 2 == 0:
                        nc.scalar.copy(dst, ps[:])
                    else:
                        nc.vector.tensor_copy(dst, ps[:])
                nc.scalar.dma_start(y[t], ot[:])

            for ci, (dt, g0, ng) in enumerate(chunks):
                if ci == 0:
                    # weights lead the SP ring: land ~8.3us, before data
                    nc.sync.dma_start(wsb[:], wm)
                xt = xpool.tile([128, ng * GQ, JB], F16, name="xt")
                nc.sync.dma_start(
                    xt[:], x[dt][:, g0 * GF : (g0 + ng) * GF]
                )

                ot = opool.tile([128, ng * GQ, JB], F16, name="ot")
                pss = [
                    pspool.tile([128, GQ, JB], F32, name="ps")
                    for _ in range(ng)
                ]
                # d-outer: one stationary load per pass. No padding: the
                # d=-1 pass would read only zeros for jb=0 and d=+1 only
                # zeros for jb=31, so those output columns are simply
                # SKIPPED (496-column matmuls) -- 2% less PE work and the
                # DMA carries pure data.
                for di, d in enumerate((0, -1, 1)):
                    for q in range(ng):
                        s0, s1 = GQ * q, GQ * (q + 1)
                        if d == 0:
                            rhs, dst = xt[:, s0:s1, :], pss[q][:]
                        elif d == -1:
                            rhs = xt[:, s0:s1, 0 : JB - 1]
                            dst = pss[q][:, :, 1:JB]
                        else:
                            rhs = xt[:, s0:s1, 1:JB]
                            dst = pss[q][:, :, 0 : JB - 1]
                        nc.tensor.matmul(
                            dst,
                            wsb[:, di, :],
                            rhs,
                            start=(di == 0),
                            stop=(di == 2),
                        )
                for q in range(ng):
                    dst = ot[:, GQ * q : GQ * (q + 1), :]
                    ylo = (g0 + q) * GQ * JB
                    if ci == last and q == ng - 1:
                        # final group: DVE-only copy, store on the idle SP
                        # ring the moment it lands -- shortest tail
                        nc.vector.tensor_copy(dst, pss[q][:])
                        nc.sync.dma_start(
                            y[dt][:, ylo : ylo + GQ * JB], dst
                        )
                        continue
                    # alternate copy engine: DVE and ACT share the load
                    if ncopy % 2 == 0:
                        nc.vector.tensor_copy(dst, pss[q][:])
                    else:
                        nc.scalar.copy(dst, pss[q][:])
                    ncopy += 1
                    if ci == last:
                        # alternate rings so the drain issues in parallel
                        eng = nc.scalar if q % 2 == 0 else nc.sync
                        eng.dma_start(y[dt][:, ylo : ylo + GQ * JB], dst)
                if ci != last:
                    ylo = g0 * GQ * JB
                    nc.scalar.dma_start(
                        y[dt][:, ylo : ylo + ng * GQ * JB], ot[:]
                    )
                # interleave offloaded tiles after their preceding chunk
                nt = dt + 1
                if g0 + ng == 4 and nt in oi and nt < ntiles:
                    offload_tile(nt)

    nc.compile()
    return nc


def get_nc(slices_per_core: int = SLICES_PER_CORE):
    if slices_per_core not in _NC_CACHE:
        _NC_CACHE[slices_per_core] = _build_nc(slices_per_core)
    return _NC_CACHE[slices_per_core]


def _pack_input(xs: np.ndarray):
    """[S, H, W] fp16 -> main tiles [S/64, 128, FREE] + offload tiles."""
    s = xs.shape[0]
    ntiles = s // TILE_SLICES
    v = np.empty((ntiles, 2, H, TILE_SLICES, JB), np.float16)
    xt = xs.reshape(ntiles, TILE_SLICES, H, W)
    v[:, 0] = xt[:, :, :, 0::2].transpose(0, 2, 1, 3)
    v[:, 1] = xt[:, :, :, 1::2].transpose(0, 2, 1, 3)
    xmain = np.ascontiguousarray(v.reshape(ntiles, 128, FREE))
    if not OFFLOAD:
        return xmain, None
    # offload tiles: partition (m, h), free (sg, w) with w zero-padded to 67
    xofs = np.zeros((len(OFFLOAD), 128, SG * WP), np.float16)
    for i, t in enumerate(OFFLOAD):
        xp = np.zeros((TILE_SLICES, H, WP), np.float16)
        xp[:, :, 2 : 2 + W] = xt[t]
        # (sg, m, h, w) -> (m, h, sg, w)
        xofs[i] = (
            xp.reshape(SG, 2, H, WP)
            .transpose(1, 2, 0, 3)
            .reshape(128, SG * WP)
        )
    return xmain, xofs


def _unpack_output(yp: np.ndarray) -> np.ndarray:
    """[S/64, 128, 64*JB] fp16 -> [S, H, W] fp16 (mixed per-tile layouts)."""
    ntiles = yp.shape[0]
    out = np.empty((ntiles, TILE_SLICES, H, W), np.float16)
    # main path: [jp, i, s, jb]
    v = yp.reshape(ntiles, 2, H, TILE_SLICES, JB)
    out[:, :, :, 0::2] = v[:, 0].transpose(0, 2, 1, 3)
    out[:, :, :, 1::2] = v[:, 1].transpose(0, 2, 1, 3)
    # offload path: [m, i, sg, w]
    for t in OFFLOAD:
        if t < ntiles:
            vo = yp[t].reshape(2, H, SG, W)
            out[t] = vo.transpose(2, 0, 1, 3).reshape(TILE_SLICES, H, W)
    return out.reshape(ntiles * TILE_SLICES, H, W)


def kernel(x: np.ndarray, kernel: np.ndarray, _trace: bool = False, **_tkw):
    xh = np.asarray(x).astype(np.float16)
    wmat = _build_wmat(kernel)
    b, c, h, w = x.shape
    xs = xh.reshape(b * c, h, w)
    spc = (b * c) // N_CORES
    nc = get_nc(spc)
    in_maps = []
    for k in range(N_CORES):
        xmain, xofs = _pack_input(xs[k * spc : (k + 1) * spc])
        m = {"x": xmain, "w": wmat}
        if xofs is not None:
            m["xo"] = xofs
        in_maps.append(m)
    res = run_bass_kernel_spmd(
        nc, in_maps, list(range(N_CORES)), trace=_trace, **_tkw
    )
    out = np.concatenate(
        [_unpack_output(res.results[k]["y"]) for k in range(N_CORES)], axis=0
    )
    result = out.reshape(b, c, h, w).astype(np.float32)
    if _trace:
        return result, res
    return result



# revision 8
# speedup vs baseline: 1.0405x; 1.0405x over previous
"""Trainium2 Bass kernel for nn_Blur: 4x4 FIR depthwise blur with pad (2,1).

out[n,c,i,j] = sum_{a,b} K[a,b] * x[n,c, i+1-a, j+1-b]   (zero-padded)

Strategy (8 NeuronCores, pure data parallelism over the 8192 (n,c) slices):
  - fp16 end-to-end on device (host converts): halves HBM traffic vs fp32.
    Quantization error ~5e-4 relative, far under the 2e-2 gate.
  - w-parity interleaved layout, partition p = 64*(w%2) + h; free dim packs
    each slice as 32 w-blocks of 2. The 16-tap conv is THREE PSUM-accumulated
    matmuls (free-dim block shifts d in {-1,0,+1}):
    lhsT_d[(jp_in,u),(jp_out,i)] = K[i-u+1, jp_out-jp_in+1-2d].
    Group-outer / d-inner order: each 512-col group's PSUM completes after
    its 3 matmuls, so copies+stores drain steadily instead of in bursts.
  - DMA: only two HW DGE rings exist (sync=qSP, scalar=qAct). Ring
    throughput is PACKET-count limited early on (~11 GB/s/engine at 1KB
    rows, ~26 at 4KB), so the FIRST transfer fuses weights+2 groups into
    one 3KB-row DMA; everything else moves in 4KB rows.
  - Startup: the HAM clock gate needs ~4.2us of CONTIGUOUS observed PE
    activity to open (1.2 -> 2.4 GHz) and a >0.5us idle gap resets the
    accumulator. Junk matmuls on an UNINITIALIZED tile (no memset, no DMA
    dependency -- garbage values are discarded via warm_out) start at the
    tensor engine's first post-preamble slot and bridge into the first
    real matmul with no gap.
  - Drain: tile 15 is split 2+2; the final two groups copy on scalar and
    vector in parallel and store down both rings simultaneously.
"""

import sys
import types

import numpy as np

import concourse.bacc as bacc
import concourse.mybir as mybir
from concourse.alu_op_type import AluOpType
from concourse.tile import TileContext
from concourse.bass_utils import run_bass_kernel_spmd


def _install_ntff_hook():
    """Best-effort shim: this image's antenv lacks axon_hooks, which the
    trace=True path of run_bass_kernel_spmd imports. Harmless if unused."""
    if "antenv.axon_hooks" in sys.modules:
        return
    try:
        sys.path.insert(0, "/root/.axon_site")
        from trn_agent_boot.trn_boot import _ntff_profile_via_ctypes

        hook = _ntff_profile_via_ctypes("/opt/axon/libaxon_pjrt.so")
        mod = types.ModuleType("antenv.axon_hooks")
        mod.get_axon_ntff_profile_hook = lambda: hook
        mod.set_axon_ntff_profile_hook = lambda h: None
        sys.modules["antenv.axon_hooks"] = mod
    except Exception:
        pass


_install_ntff_hook()

N_CORES = 8
B, C, H, W = 32, 256, 64, 64
NSLICES = B * C                      # 8192
SLICES_PER_CORE = NSLICES // N_CORES  # 1024
TILE_SLICES = 64                     # slices per full SBUF tile
JB = W // 2                          # 32 w-blocks of 2 per slice
FREE = TILE_SLICES * JB              # 2048: NO padding (edge-skip matmuls)
GQ = 16                              # slices per PSUM group (N = 16*32 = 512)
GF = GQ * JB                         # free columns per group = 512
WP = W + 3                           # offload path: 2 left + 1 right zero
SG = TILE_SLICES // 2                # offload path: s-groups per member
F16 = mybir.dt.float16
F32 = mybir.dt.float32

# Separable DVE/GPS offload of whole tiles (W-conv on vector engines plus a
# single h-band PE pass) -- tiles listed here skip the 3-pass matmul path.
OFFLOAD = ()
WARMUP_MMS = 12                      # 128-col junk matmuls: ~7.3us -> ~9.9us

_NC_CACHE = {}


def _build_wmat(K: np.ndarray) -> np.ndarray:
    """[128, 4*128] fp16: lhsT stack [d=0, d=-1, d=+1, h-band k1/16]."""
    K = np.asarray(K, np.float32)
    wmat = np.zeros((4, 128, 128), np.float32)
    for di, d in enumerate((0, -1, 1)):
        L = wmat[di]
        for jpi in range(2):
            for jpo in range(2):
                b = jpo - jpi + 1 - 2 * d
                if not (0 <= b < 4):
                    continue
                for i in range(H):
                    for a in range(4):
                        u = i + 1 - a
                        if 0 <= u < H:
                            L[64 * jpi + u, 64 * jpo + i] += K[a, b]
    # h-band for the separable path: lhsT[u+64m, i+64m] = k1[i-u+1]/16
    k1 = np.array([1.0, 3.0, 3.0, 1.0], np.float32) / 16.0
    T = np.zeros((H, H), np.float32)
    for i in range(H):
        for a in range(4):
            u = i + 1 - a
            if 0 <= u < H:
                T[u, i] += k1[a]
    wmat[3, :H, :H] = T
    wmat[3, H:, H:] = T
    # [d, k, m] -> [k, (d m)] so the DMA is one contiguous run per partition
    return np.ascontiguousarray(
        wmat.transpose(1, 0, 2).reshape(128, 4 * 128)
    ).astype(np.float16)


def _build_nc(slices_per_core: int = SLICES_PER_CORE):
    ntiles = slices_per_core // TILE_SLICES
    nc = bacc.Bacc("TRN2", target_bir_lowering=False, debug=False)
    # x0 fuses the 512-col weight stack with tile 0 (2048 cols): the head
    # of the SP ring moves 3KB rows instead of 1KB ones.
    x0 = nc.dram_tensor("x0", [128, 512 + FREE], F16, kind="ExternalInput").ap()
    x = nc.dram_tensor(
        "x", [ntiles - 1, 128, FREE], F16, kind="ExternalInput"
    ).ap()
    xo = (
        nc.dram_tensor(
            "xo", [len(OFFLOAD), 128, SG * WP], F16, kind="ExternalInput"
        ).ap()
        if OFFLOAD
        else None
    )
    y = nc.dram_tensor(
        "y", [ntiles, 128, TILE_SLICES * JB], F16, kind="ExternalOutput"
    ).ap()
    # sink for the PE warm-up matmuls (kept alive so DCE can't drop them)
    warm_out = nc.dram_tensor("warm", [128, 4], F32, kind="ExternalOutput").ap()

    # main-path chunk list: ("t0",) = fused weights+groups 0-1 of tile 0;
    # ("x0b",) = groups 2-3 of tile 0; then (tile, g0, ng) 4-group chunks;
    # tile 15 split 2+2 for a two-ring parallel drain.
    chunks = [("t0",), ("x0b",)]
    chunks += [(t, 0, 4) for t in range(1, ntiles - 1) if t not in OFFLOAD]
    if (ntiles - 1) not in OFFLOAD:
        chunks += [(ntiles - 1, 0, 2), (ntiles - 1, 2, 2)]
    last = len(chunks) - 1

    with TileContext(nc) as tc:
        with (
            tc.tile_pool(name="wpool", bufs=1) as wpool,
            tc.tile_pool(name="xpool", bufs=8) as xpool,
            tc.tile_pool(name="vpool", bufs=4) as vpool,
            tc.tile_pool(name="opool", bufs=6) as opool,
            tc.tile_pool(name="pspool", bufs=8, space="PSUM") as pspool,
        ):
            # t0 holds [weights(16 blocks) | tile0 groups 0-1 (32 blocks)],
            # each block = 32 cols. One 3KB-row DMA brings all of it.
            t0 = wpool.tile([128, 48, 32], F16, name="t0")
            nc.sync.dma_start(t0[:], x0[:, 0 : 48 * 32])

            def wap(di):
                return t0[:, 4 * di : 4 * di + 4, :]

            # HAM warm-up: a tiny [128,128] memset on DVE (~150ns at its
            # first post-preamble slot) unblocks a run of 128-col junk
            # matmuls that keep the PE busy from ~7.3us until the t0 DMA
            # lands (~9.9us) -- the clock-gate accumulator never resets.
            wjunk = wpool.tile([128, 128], F16, name="wjunk")
            nc.vector.memset(wjunk[:], 0.0)
            wscratch = wpool.tile([128, 4], F32, name="wscratch")
            wps = pspool.tile([128, 128], F32, name="wps", tag="ps")
            for r in range(WARMUP_MMS):
                nc.tensor.matmul(
                    wps[:],
                    wjunk[:],
                    wjunk[:],
                    start=(r == 0),
                    stop=(r == WARMUP_MMS - 1),
                )
            nc.vector.tensor_copy(wscratch[:], wps[:, 0:4])
            nc.scalar.dma_start(warm_out, wscratch[:])

            oi = {t: i for i, t in enumerate(OFFLOAD)}
            ncopy = 0

            def offload_tile(t):
                """Separable path: W-conv on DVE/GPS, one h-band PE pass."""
                xt = xpool.tile([128, SG, WP], F16, name="xof")
                nc.sync.dma_start(xt[:], xo[oi[t]])
                t1 = vpool.tile([128, SG, W], F16, name="t1")
                t2 = vpool.tile([128, SG, W], F16, name="t2")
                y2 = vpool.tile([128, SG, W], F16, name="y2")
                nc.gpsimd.tensor_tensor(
                    t1[:], xt[:, :, 0:W], xt[:, :, 3 : 3 + W], AluOpType.add
                )
                nc.gpsimd.tensor_tensor(
                    t2[:], xt[:, :, 1 : 1 + W], xt[:, :, 2 : 2 + W],
                    AluOpType.add,
                )
                nc.vector.scalar_tensor_tensor(
                    y2[:], t2[:], 3.0, t1[:],
                    op0=AluOpType.mult, op1=AluOpType.add,
                )
                ot = opool.tile([128, SG, W], F16, name="ot")
                for q in range(4):
                    ps = pspool.tile([128, GQ * JB], F32, name="ps")
                    nc.tensor.matmul(
                        ps[:], wap(3), y2[:, 8 * q : 8 * (q + 1), :],
                        start=True, stop=True,
                    )
                    dst = ot[:, 8 * q : 8 * (q + 1), :]
                    if q % 2 == 0:
                        nc.scalar.copy(dst, ps[:])
                    else:
                        nc.vector.tensor_copy(dst, ps[:])
                nc.scalar.dma_start(y[t], ot[:])

            for ci, ch in enumerate(chunks):
                if ch[0] == "t0":
                    dt, g0, ng = 0, 0, 2
                    grp = lambda g: t0[:, 16 + 16 * g : 32 + 16 * g, :]
                    grpl = lambda g: t0[:, 16 + 16 * g : 32 + 16 * g, 0 : JB - 1]
                    grpr = lambda g: t0[:, 16 + 16 * g : 32 + 16 * g, 1:JB]
                elif ch[0] == "x0b":
                    dt, g0, ng = 0, 2, 2
                    xt = xpool.tile([128, ng * GQ, JB], F16, name="xt")
                    nc.sync.dma_start(xt[:], x0[:, 512 + g0 * GF :])
                    grp = lambda g: xt[:, GQ * g : GQ * (g + 1), :]
                    grpl = lambda g: xt[:, GQ * g : GQ * (g + 1), 0 : JB - 1]
                    grpr = lambda g: xt[:, GQ * g : GQ * (g + 1), 1:JB]
                else:
                    dt, g0, ng = ch
                    xt = xpool.tile([128, ng * GQ, JB], F16, name="xt")
                    nc.sync.dma_start(
                        xt[:], x[dt - 1][:, g0 * GF : (g0 + ng) * GF]
                    )
                    grp = lambda g: xt[:, GQ * g : GQ * (g + 1), :]
                    grpl = lambda g: xt[:, GQ * g : GQ * (g + 1), 0 : JB - 1]
                    grpr = lambda g: xt[:, GQ * g : GQ * (g + 1), 1:JB]

                ot = opool.tile([128, ng * GQ, JB], F16, name="ot")
                # group-outer, d-inner: group q's PSUM is complete after its
                # own 3 matmuls; its copy runs while the PE streams q+1.
                # No padding: d=-1 skips output col jb=0, d=+1 skips jb=31.
                for q in range(ng):
                    ps = pspool.tile([128, GQ, JB], F32, name="ps")
                    nc.tensor.matmul(
                        ps[:], wap(0), grp(q), start=True, stop=False
                    )
                    nc.tensor.matmul(
                        ps[:, :, 1:JB], wap(1), grpl(q), start=False, stop=False
                    )
                    nc.tensor.matmul(
                        ps[:, :, 0 : JB - 1], wap(2), grpr(q),
                        start=False, stop=True,
                    )
                    dst = ot[:, GQ * q : GQ * (q + 1), :]
                    ylo = (g0 + q) * GQ * JB
                    if ci == last:
                        # final two groups: scalar/vector copies in
                        # parallel, stores down both rings at once
                        if q == 0:
                            nc.scalar.copy(dst, ps[:])
                            nc.scalar.dma_start(
                                y[dt][:, ylo : ylo + GQ * JB], dst
                            )
                        else:
                            nc.vector.tensor_copy(dst, ps[:])
                            nc.sync.dma_start(
                                y[dt][:, ylo : ylo + GQ * JB], dst
                            )
                        continue
                    # alternate copy engine: DVE and ACT share the load
                    if ncopy % 2 == 0:
                        nc.vector.tensor_copy(dst, ps[:])
                    else:
                        nc.scalar.copy(dst, ps[:])
                    ncopy += 1
                if ci != last:
                    ylo = g0 * GQ * JB
                    nc.scalar.dma_start(
                        y[dt][:, ylo : ylo + ng * GQ * JB], ot[:]
                    )
                # interleave offloaded tiles after their preceding chunk
                if ch[0] not in ("t0", "x0b"):
                    nt = dt + 1
                    if g0 + ng == 4 and nt in oi and nt < ntiles:
                        offload_tile(nt)

    nc.compile()
    return nc


def get_nc(slices_per_core: int = SLICES_PER_CORE):
    if slices_per_core not in _NC_CACHE:
        _NC_CACHE[slices_per_core] = _build_nc(slices_per_core)
    return _NC_CACHE[slices_per_core]


def _pack_input(xs: np.ndarray):
    """[S, H, W] fp16 -> main tiles [S/64, 128, FREE] + offload tiles."""
    s = xs.shape[0]
    ntiles = s // TILE_SLICES
    v = np.empty((ntiles, 2, H, TILE_SLICES, JB), np.float16)
    xt = xs.reshape(ntiles, TILE_SLICES, H, W)
    v[:, 0] = xt[:, :, :, 0::2].transpose(0, 2, 1, 3)
    v[:, 1] = xt[:, :, :, 1::2].transpose(0, 2, 1, 3)
    xmain = np.ascontiguousarray(v.reshape(ntiles, 128, FREE))
    if not OFFLOAD:
        return xmain, None
    # offload tiles: partition (m, h), free (sg, w) with w zero-padded to 67
    xofs = np.zeros((len(OFFLOAD), 128, SG * WP), np.float16)
    for i, t in enumerate(OFFLOAD):
        xp = np.zeros((TILE_SLICES, H, WP), np.float16)
        xp[:, :, 2 : 2 + W] = xt[t]
        # (sg, m, h, w) -> (m, h, sg, w)
        xofs[i] = (
            xp.reshape(SG, 2, H, WP)
            .transpose(1, 2, 0, 3)
            .reshape(128, SG * WP)
        )
    return xmain, xofs


def _unpack_output(yp: np.ndarray) -> np.ndarray:
    """[S/64, 128, 64*JB] fp16 -> [S, H, W] fp16 (mixed per-tile layouts)."""
    ntiles = yp.shape[0]
    out = np.empty((ntiles, TILE_SLICES, H, W), np.float16)
    # main path: [jp, i, s, jb]
    v = yp.reshape(ntiles, 2, H, TILE_SLICES, JB)
    out[:, :, :, 0::2] = v[:, 0].transpose(0, 2, 1, 3)
    out[:, :, :, 1::2] = v[:, 1].transpose(0, 2, 1, 3)
    # offload path: [m, i, sg, w]
    for t in OFFLOAD:
        if t < ntiles:
            vo = yp[t].reshape(2, H, SG, W)
            out[t] = vo.transpose(2, 0, 1, 3).reshape(TILE_SLICES, H, W)
    return out.reshape(ntiles * TILE_SLICES, H, W)


def kernel(x: np.ndarray, kernel: np.ndarray, _trace: bool = False, **_tkw):
    xh = np.asarray(x).astype(np.float16)
    wmat = _build_wmat(kernel)
    b, c, h, w = x.shape
    xs = xh.reshape(b * c, h, w)
    spc = (b * c) // N_CORES
    nc = get_nc(spc)
    in_maps = []
    for k in range(N_CORES):
        xmain, xofs = _pack_input(xs[k * spc : (k + 1) * spc])
        m = {
            "x0": np.ascontiguousarray(
                np.concatenate([wmat, xmain[0]], axis=1)
            ),
            "x": xmain[1:],
        }
        if xofs is not None:
            m["xo"] = xofs
        in_maps.append(m)
    res = run_bass_kernel_spmd(
        nc, in_maps, list(range(N_CORES)), trace=_trace, **_tkw
    )
    out = np.concatenate(
        [_unpack_output(res.results[k]["y"]) for k in range(N_CORES)], axis=0
    )
    result = out.reshape(b, c, h, w).astype(np.float32)
    if _trace:
        return result, res
    return result


# revision 14
# speedup vs baseline: 1.0716x; 1.0299x over previous
"""Trainium2 Bass kernel for nn_Blur: 4x4 FIR depthwise blur with pad (2,1).

out[n,c,i,j] = sum_{a,b} K[a,b] * x[n,c, i+1-a, j+1-b]   (zero-padded)

Strategy (8 NeuronCores, pure data parallelism over the 8192 (n,c) slices):
  - fp16 end-to-end on device (host converts): halves HBM traffic vs fp32.
    Quantization error ~5e-4 relative, far under the 2e-2 gate.
  - w-parity interleaved layout, partition p = 64*(w%2) + h; free dim packs
    each slice as 32 w-blocks of 2. The 16-tap conv is THREE PSUM-accumulated
    matmuls (free-dim block shifts d in {-1,0,+1}):
    lhsT_d[(jp_in,u),(jp_out,i)] = K[i-u+1, jp_out-jp_in+1-2d].
    Group-outer / d-inner order: each 512-col group's PSUM completes after
    its 3 matmuls, so copies+stores drain steadily instead of in bursts.
  - DMA: only two HW DGE rings exist (sync=qSP, scalar=qAct). Ring
    throughput is PACKET-count limited early on (~11 GB/s/engine at 1KB
    rows, ~26 at 4KB), so the FIRST transfer fuses weights+2 groups into
    one 3KB-row DMA; everything else moves in 4KB rows.
  - Startup: the HAM clock gate needs ~4.2us of CONTIGUOUS observed PE
    activity to open (1.2 -> 2.4 GHz) and a >0.5us idle gap resets the
    accumulator. Junk matmuls on an UNINITIALIZED tile (no memset, no DMA
    dependency -- garbage values are discarded via warm_out) start at the
    tensor engine's first post-preamble slot and bridge into the first
    real matmul with no gap.
  - Drain: tile 15 is split 2+2; the final two groups copy on scalar and
    vector in parallel and store down both rings simultaneously.
"""

import sys
import types

import numpy as np

import concourse.bacc as bacc
import concourse.mybir as mybir
from concourse.alu_op_type import AluOpType
from concourse.tile import TileContext
from concourse.bass_utils import run_bass_kernel_spmd


def _install_ntff_hook():
    """Best-effort shim: this image's antenv lacks axon_hooks, which the
    trace=True path of run_bass_kernel_spmd imports. Harmless if unused."""
    if "antenv.axon_hooks" in sys.modules:
        return
    try:
        sys.path.insert(0, "/root/.axon_site")
        from trn_agent_boot.trn_boot import _ntff_profile_via_ctypes

        hook = _ntff_profile_via_ctypes("/opt/axon/libaxon_pjrt.so")
        mod = types.ModuleType("antenv.axon_hooks")
        mod.get_axon_ntff_profile_hook = lambda: hook
        mod.set_axon_ntff_profile_hook = lambda h: None
        sys.modules["antenv.axon_hooks"] = mod
    except Exception:
        pass


_install_ntff_hook()

N_CORES = 8
B, C, H, W = 32, 256, 64, 64
NSLICES = B * C                      # 8192
SLICES_PER_CORE = NSLICES // N_CORES  # 1024
TILE_SLICES = 64                     # slices per full SBUF tile
JB = W // 2                          # 32 w-blocks of 2 per slice
FREE = TILE_SLICES * JB              # 2048: NO padding (edge-skip matmuls)
GQ = 16                              # slices per PSUM group (N = 16*32 = 512)
GF = GQ * JB                         # free columns per group = 512
WP = W + 3                           # offload path: 2 left + 1 right zero
SG = TILE_SLICES // 2                # offload path: s-groups per member
F16 = mybir.dt.float16
F32 = mybir.dt.float32

# Separable DVE/GPS offload of whole tiles (W-conv on vector engines plus a
# single h-band PE pass) -- tiles listed here skip the 3-pass matmul path.
OFFLOAD = ()
WARMUP_MMS = 28                      # 128-col junk matmuls: ~7.6us -> ~10.6us

_NC_CACHE = {}


def _build_wmat(K: np.ndarray) -> np.ndarray:
    """[128, 4*128] fp16: lhsT stack [d=0, d=-1, d=+1, h-band k1/16]."""
    K = np.asarray(K, np.float32)
    wmat = np.zeros((4, 128, 128), np.float32)
    for di, d in enumerate((0, -1, 1)):
        L = wmat[di]
        for jpi in range(2):
            for jpo in range(2):
                b = jpo - jpi + 1 - 2 * d
                if not (0 <= b < 4):
                    continue
                for i in range(H):
                    for a in range(4):
                        u = i + 1 - a
                        if 0 <= u < H:
                            L[64 * jpi + u, 64 * jpo + i] += K[a, b]
    # h-band for the separable path: lhsT[u+64m, i+64m] = k1[i-u+1]/16
    k1 = np.array([1.0, 3.0, 3.0, 1.0], np.float32) / 16.0
    T = np.zeros((H, H), np.float32)
    for i in range(H):
        for a in range(4):
            u = i + 1 - a
            if 0 <= u < H:
                T[u, i] += k1[a]
    wmat[3, :H, :H] = T
    wmat[3, H:, H:] = T
    # [d, k, m] -> [k, (d m)] so the DMA is one contiguous run per partition
    return np.ascontiguousarray(
        wmat.transpose(1, 0, 2).reshape(128, 4 * 128)
    ).astype(np.float16)


def _build_nc(slices_per_core: int = SLICES_PER_CORE):
    ntiles = slices_per_core // TILE_SLICES
    nc = bacc.Bacc("TRN2", target_bir_lowering=False, debug=False)
    x = nc.dram_tensor(
        "x", [ntiles, 128, FREE], F16, kind="ExternalInput"
    ).ap()
    wm = nc.dram_tensor("w", [128, 4 * 128], F16, kind="ExternalInput").ap()
    xo = (
        nc.dram_tensor(
            "xo", [len(OFFLOAD), 128, SG * WP], F16, kind="ExternalInput"
        ).ap()
        if OFFLOAD
        else None
    )
    y = nc.dram_tensor(
        "y", [ntiles, 128, TILE_SLICES * JB], F16, kind="ExternalOutput"
    ).ap()
    # sink for the PE warm-up matmuls (kept alive so DCE can't drop them)
    warm_out = nc.dram_tensor("warm", [128, 4], F32, kind="ExternalOutput").ap()

    # main-path chunk list: tile 0 split 2+2 (2KB-row first transfer is
    # the empirically fastest first landing -- 1KB and 3KB rows both lose
    # to the ring's cold-start shape); tile 15 split 2+2 so the last two
    # stores are 2KB-row 2-group transfers down both rings in parallel.
    chunks = [(0, 0, 2), (0, 2, 2)]
    chunks += [(t, 0, 4) for t in range(1, ntiles - 1) if t not in OFFLOAD]
    if (ntiles - 1) not in OFFLOAD:
        chunks += [(ntiles - 1, 0, 2), (ntiles - 1, 2, 2)]
    last = len(chunks) - 1

    with TileContext(nc) as tc:
        with (
            tc.tile_pool(name="wpool", bufs=1) as wpool,
            tc.tile_pool(name="xpool", bufs=8) as xpool,
            tc.tile_pool(name="vpool", bufs=4) as vpool,
            tc.tile_pool(name="opool", bufs=6) as opool,
            tc.tile_pool(name="pspool", bufs=8, space="PSUM") as pspool,
        ):
            # weight tile: rides the SP ring right behind tile 0's first
            # two groups (the baseline-measured fastest start: both sems
            # land ~10.7us).
            wsb = wpool.tile([128, 4, 128], F16, name="wsb")

            def wap(di):
                return wsb[:, di, :]

            # HAM warm-up: a tiny [128,128] memset on DVE (~150ns at its
            # first post-preamble slot) unblocks a run of 128-col junk
            # matmuls that keep the PE busy from ~7.3us until the t0 DMA
            # lands (~9.9us) -- the clock-gate accumulator never resets.
            wjunk = wpool.tile([128, 128], F16, name="wjunk")
            nc.vector.memset(wjunk[:], 0.0)
            wscratch = wpool.tile([128, 4], F32, name="wscratch")
            wps = pspool.tile([128, 128], F32, name="wps", tag="ps")
            for r in range(WARMUP_MMS):
                nc.tensor.matmul(
                    wps[:],
                    wjunk[:],
                    wjunk[:],
                    start=(r == 0),
                    stop=(r == WARMUP_MMS - 1),
                )
            nc.vector.tensor_copy(wscratch[:], wps[:, 0:4])
            nc.scalar.dma_start(warm_out, wscratch[:])

            oi = {t: i for i, t in enumerate(OFFLOAD)}
            ncopy = 0

            def offload_tile(t):
                """Separable path: W-conv on DVE/GPS, one h-band PE pass."""
                xt = xpool.tile([128, SG, WP], F16, name="xof")
                nc.sync.dma_start(xt[:], xo[oi[t]])
                t1 = vpool.tile([128, SG, W], F16, name="t1")
                t2 = vpool.tile([128, SG, W], F16, name="t2")
                y2 = vpool.tile([128, SG, W], F16, name="y2")
                nc.gpsimd.tensor_tensor(
                    t1[:], xt[:, :, 0:W], xt[:, :, 3 : 3 + W], AluOpType.add
                )
                nc.gpsimd.tensor_tensor(
                    t2[:], xt[:, :, 1 : 1 + W], xt[:, :, 2 : 2 + W],
                    AluOpType.add,
                )
                nc.vector.scalar_tensor_tensor(
                    y2[:], t2[:], 3.0, t1[:],
                    op0=AluOpType.mult, op1=AluOpType.add,
                )
                ot = opool.tile([128, SG, W], F16, name="ot")
                for q in range(4):
                    ps = pspool.tile([128, GQ * JB], F32, name="ps")
                    nc.tensor.matmul(
                        ps[:], wap(3), y2[:, 8 * q : 8 * (q + 1), :],
                        start=True, stop=True,
                    )
                    dst = ot[:, 8 * q : 8 * (q + 1), :]
                    if q % 2 == 0:
                        nc.scalar.copy(dst, ps[:])
                    else:
                        nc.vector.tensor_copy(dst, ps[:])
                nc.scalar.dma_start(y[t], ot[:])

            for ci, (dt, g0, ng) in enumerate(chunks):
                xt = xpool.tile([128, ng * GQ, JB], F16, name="xt")
                nc.sync.dma_start(
                    xt[:], x[dt][:, g0 * GF : (g0 + ng) * GF]
                )
                if ci == 0:
                    # weights ride the SP ring second: land with chunk 0
                    nc.sync.dma_start(wsb[:], wm)
                grp = lambda g: xt[:, GQ * g : GQ * (g + 1), :]
                grpl = lambda g: xt[:, GQ * g : GQ * (g + 1), 0 : JB - 1]
                grpr = lambda g: xt[:, GQ * g : GQ * (g + 1), 1:JB]

                ot = opool.tile([128, ng * GQ, JB], F16, name="ot")
                # group-outer, d-inner: group q's PSUM is complete after its
                # own 3 matmuls; its copy runs while the PE streams q+1.
                # No padding: d=-1 skips output col jb=0, d=+1 skips jb=31.
                tailc = ci >= last - 1
                for q in range(ng):
                    ps = pspool.tile([128, GQ, JB], F32, name="ps")
                    nc.tensor.matmul(
                        ps[:], wap(0), grp(q), start=True, stop=False
                    )
                    nc.tensor.matmul(
                        ps[:, :, 1:JB], wap(1), grpl(q), start=False, stop=False
                    )
                    nc.tensor.matmul(
                        ps[:, :, 0 : JB - 1], wap(2), grpr(q),
                        start=False, stop=True,
                    )
                    dst = ot[:, GQ * q : GQ * (q + 1), :]
                    if tailc:
                        # last two chunks: copies pinned scalar/vector so
                        # the pair runs in parallel
                        if q == 0:
                            nc.scalar.copy(dst, ps[:])
                        else:
                            nc.vector.tensor_copy(dst, ps[:])
                        continue
                    # alternate copy engine: DVE and ACT share the load
                    if ncopy % 2 == 0:
                        nc.vector.tensor_copy(dst, ps[:])
                    else:
                        nc.scalar.copy(dst, ps[:])
                    ncopy += 1
                ylo = g0 * GQ * JB
                if not tailc:
                    nc.scalar.dma_start(
                        y[dt][:, ylo : ylo + ng * GQ * JB], ot[:]
                    )
                else:
                    # 2-group 2KB-row stores, one per ring, in parallel
                    eng = nc.scalar if ci == last - 1 else nc.sync
                    eng.dma_start(
                        y[dt][:, ylo : ylo + ng * GQ * JB], ot[:]
                    )
                # interleave offloaded tiles after their preceding chunk
                nt = dt + 1
                if g0 + ng == 4 and nt in oi and nt < ntiles:
                    offload_tile(nt)

    nc.compile()
    return nc


def get_nc(slices_per_core: int = SLICES_PER_CORE):
    if slices_per_core not in _NC_CACHE:
        _NC_CACHE[slices_per_core] = _build_nc(slices_per_core)
    return _NC_CACHE[slices_per_core]


def _pack_input(xs: np.ndarray):
    """[S, H, W] fp16 -> main tiles [S/64, 128, FREE] + offload tiles."""
    s = xs.shape[0]
    ntiles = s // TILE_SLICES
    v = np.empty((ntiles, 2, H, TILE_SLICES, JB), np.float16)
    xt = xs.reshape(ntiles, TILE_SLICES, H, W)
    v[:, 0] = xt[:, :, :, 0::2].transpose(0, 2, 1, 3)
    v[:, 1] = xt[:, :, :, 1::2].transpose(0, 2, 1, 3)
    xmain = np.ascontiguousarray(v.reshape(ntiles, 128, FREE))
    if not OFFLOAD:
        return xmain, None
    # offload tiles: partition (m, h), free (sg, w) with w zero-padded to 67
    xofs = np.zeros((len(OFFLOAD), 128, SG * WP), np.float16)
    for i, t in enumerate(OFFLOAD):
        xp = np.zeros((TILE_SLICES, H, WP), np.float16)
        xp[:, :, 2 : 2 + W] = xt[t]
        # (sg, m, h, w) -> (m, h, sg, w)
        xofs[i] = (
            xp.reshape(SG, 2, H, WP)
            .transpose(1, 2, 0, 3)
            .reshape(128, SG * WP)
        )
    return xmain, xofs


def _unpack_output(yp: np.ndarray) -> np.ndarray:
    """[S/64, 128, 64*JB] fp16 -> [S, H, W] fp16 (mixed per-tile layouts)."""
    ntiles = yp.shape[0]
    out = np.empty((ntiles, TILE_SLICES, H, W), np.float16)
    # main path: [jp, i, s, jb]
    v = yp.reshape(ntiles, 2, H, TILE_SLICES, JB)
    out[:, :, :, 0::2] = v[:, 0].transpose(0, 2, 1, 3)
    out[:, :, :, 1::2] = v[:, 1].transpose(0, 2, 1, 3)
    # offload path: [m, i, sg, w]
    for t in OFFLOAD:
        if t < ntiles:
            vo = yp[t].reshape(2, H, SG, W)
            out[t] = vo.transpose(2, 0, 1, 3).reshape(TILE_SLICES, H, W)
    return out.reshape(ntiles * TILE_SLICES, H, W)


def kernel(x: np.ndarray, kernel: np.ndarray, _trace: bool = False, **_tkw):
    xh = np.asarray(x).astype(np.float16)
    wmat = _build_wmat(kernel)
    b, c, h, w = x.shape
    xs = xh.reshape(b * c, h, w)
    spc = (b * c) // N_CORES
    nc = get_nc(spc)
    in_maps = []
    for k in range(N_CORES):
        xmain, xofs = _pack_input(xs[k * spc : (k + 1) * spc])
        m = {"x": xmain, "w": wmat}
        if xofs is not None:
            m["xo"] = xofs
        in_maps.append(m)
    res = run_bass_kernel_spmd(
        nc, in_maps, list(range(N_CORES)), trace=_trace, **_tkw
    )
    out = np.concatenate(
        [_unpack_output(res.results[k]["y"]) for k in range(N_CORES)], axis=0
    )
    result = out.reshape(b, c, h, w).astype(np.float32)
    if _trace:
        return result, res
    return result
